# revision 5
# baseline (speedup 1.0000x reference)
"""Multi-head attention (B=2, S=2048, d_model=1024, 16 heads) on 8 TRN2 cores.

Sharding: core c = (batch b, head-group g) with b = c // 4, g = c % 4; each core
owns 4 heads of one batch element (Megatron-style column split of w_q/w_k/w_v,
row split of w_o, data parallel over batch).

Device computes, per core:
  - eT[h, k, q]  = exp(scores[q, k] / 8)   (TRANSPOSED, unnormalized)
  - r[h, q]      = sum_k exp(...)          (softmax denominators, via a
                                            ones-column folded into the
                                            context matmul)
  - outp[q, :]   = sum_h (exp/r) @ V_h @ W_o_h.T   (normalized, no bias)

The transposed score layout lets the context matmul consume exp tiles straight
from SBUF with k on partitions (no on-chip transpose, single exp pass).  The
host applies 1/r and transposes eT into the required attn_weights layout while
assembling the full outputs (that pass is needed anyway to gather shards).

All matmuls run as float32r (full-rate); everything else is fp32.
"""

import numpy as np
from contextlib import ExitStack

import concourse.bass as bass
import concourse.bacc as bacc
import concourse.tile as tile
from concourse import mybir
from concourse.bass_utils import run_bass_kernel_spmd

F32 = mybir.dt.float32
F32R = mybir.dt.float32r

D_MODEL = 1024
NUM_HEADS = 16
HEAD_DIM = D_MODEL // NUM_HEADS
B_FULL = 2
S_FULL = 2048
N_CORES = 8
HG = 4                      # heads per core
SCALE = float(np.sqrt(HEAD_DIM))

# stash of the most recent BassKernelResults (test.py reads exec_time_ns)
LAST_RESULTS = None


def _r(ap):
    return ap.bitcast(F32R)


def build_mha_kernel(nc, S, D, heads, dh):
    """Emit the per-core MHA program. Shapes:
      xqT/xkT/xvT: [D, S]   (input activations, pre-transposed on host)
      wqT/wkT/wvT: [D, G]   (G = heads*dh head-group slice, pre-transposed)
      woT:         [G, D]
      eT:  [heads, S, S]  out   (exp scores, transposed per head: [k, q])
      r:   [heads, S]     out   (softmax denominators)
      outp:[S, D]         out   (output-projection partial, normalized)
    """
    P = 128
    QB = 512                 # q-block width (psum free dim)
    G = heads * dh
    KC = S // P              # k chunks of 128
    DC = D // P              # d chunks of 128
    NQB = S // QB            # q blocks
    NSB = S // QB            # s blocks for q/k projections
    NT = G // P              # number of 128-row tiles for qT/kT
    assert G % P == 0 and S % QB == 0 and D % P == 0

    ap = {}
    for name, shape in [("xqT", [D, S]), ("xkT", [D, S]), ("xvT", [D, S]),
                        ("wqT", [D, G]), ("wkT", [D, G]), ("wvT", [D, G]),
                        ("woT", [G, D])]:
        ap[name] = nc.dram_tensor(name, shape, F32R, kind="ExternalInput").ap()
    eT_ap = nc.dram_tensor("eT", [heads, S, S], F32R, kind="ExternalOutput").ap()
    r_ap = nc.dram_tensor("r", [heads, S], F32R, kind="ExternalOutput").ap()
    outp_ap = nc.dram_tensor("outp", [S, D], F32, kind="ExternalOutput").ap()

    with tile.TileContext(nc) as tc, ExitStack() as ctx:
        # ---- persistent pools (live across phases) ----
        qk_pool = ctx.enter_context(tc.tile_pool(name="qk", bufs=1))
        v_pool = ctx.enter_context(tc.tile_pool(name="vaug", bufs=1))
        wo_pool = ctx.enter_context(tc.tile_pool(name="wo", bufs=1))
        ctx_pool = ctx.enter_context(tc.tile_pool(name="ctx", bufs=1))

        qt = [qk_pool.tile([P, S], F32R, tag=f"qt{m}", name=f"qt{m}") for m in range(NT)]
        kt = [qk_pool.tile([P, S], F32R, tag=f"kt{m}", name=f"kt{m}") for m in range(NT)]
        v_aug = v_pool.tile([P, KC, heads, dh + 1], F32R)
        wo = [wo_pool.tile([dh, D], F32R, tag=f"wo{h}", name=f"wo{h}") for h in range(heads)]
        ctxt = [ctx_pool.tile([dh + 1, S], F32R, tag=f"ctx{h}", name=f"ctx{h}") for h in range(heads)]

        for h in range(heads):
            nc.sync.dma_start(wo[h][:], ap["woT"][h * dh:(h + 1) * dh, :])
        # ones column for the context-matmul row-sum trick (memset cannot
        # write fp32r, so memset an fp32 tile and let a DVE copy round it)
        ones = v_pool.tile([P, KC * heads], F32)
        nc.gpsimd.memset(ones[:], 1.0)
        nc.vector.tensor_copy(
            v_aug[:, :, :, dh:dh + 1],
            ones[:].rearrange("p (c h o) -> p c h o", c=KC, h=heads, o=1))

        # ---- phase 1: projections ----
        with ExitStack() as pctx:
            w_pool = pctx.enter_context(tc.tile_pool(name="wqkv", bufs=1))
            x_pool = pctx.enter_context(tc.tile_pool(name="xt", bufs=10))
            pj_psum = pctx.enter_context(
                tc.tile_pool(name="pjpsum", bufs=4, space="PSUM"))

            wts = {}
            for wname in ("wqT", "wkT", "wvT"):
                wt = w_pool.tile([P, DC, G], F32R, tag=wname, name=wname)
                nc.sync.dma_start(
                    wt[:], ap[wname].rearrange("(c p) n -> p c n", p=P))
                wts[wname] = wt

            def load_x(xname):
                tiles = []
                for c in range(DC):
                    t = x_pool.tile([P, S], F32R, tag="xc", name="xc")
                    nc.sync.dma_start(t[:], ap[xname][c * P:(c + 1) * P, :])
                    tiles.append(t)
                return tiles

            # q/k projections -> qt/kt in [dims, s] layout
            for wname, dst in (("wqT", qt), ("wkT", kt)):
                xts = load_x("xqT" if wname == "wqT" else "xkT")
                for m in range(NT):
                    for jb in range(NSB):
                        ps = pj_psum.tile([P, QB], F32, tag="pj", name="pj")
                        for c in range(DC):
                            nc.tensor.matmul(
                                ps[:],
                                wts[wname][:, c, m * P:(m + 1) * P],
                                xts[c][:, jb * QB:(jb + 1) * QB],
                                start=(c == 0), stop=(c == DC - 1))
                        nc.vector.tensor_copy(
                            dst[m][:, jb * QB:(jb + 1) * QB], ps[:])

            # v projection -> v_aug in [s, dims] layout (natural)
            xts = load_x("xvT")
            for i in range(KC):
                ps = pj_psum.tile([P, G], F32, tag="pv", name="pv")
                for c in range(DC):
                    nc.tensor.matmul(
                        ps[:],
                        xts[c][:, i * P:(i + 1) * P],
                        wts["wvT"][:, c, :],
                        start=(c == 0), stop=(c == DC - 1))
                for h in range(heads):
                    nc.vector.tensor_copy(
                        v_aug[:, i, h, 0:dh], ps[:, h * dh:(h + 1) * dh])

        # ---- phase 2: attention ----
        with ExitStack() as actx:
            e_pool = actx.enter_context(tc.tile_pool(name="et", bufs=2))
            s_psum = actx.enter_context(
                tc.tile_pool(name="spsum", bufs=2, space="PSUM"))
            c_psum = actx.enter_context(
                tc.tile_pool(name="cpsum", bufs=2, space="PSUM"))

            for h in range(heads):
                m, prow = h // 2, (h % 2) * dh if dh < P else 0
                ktile, qtile = kt[m], qt[m]
                for j in range(NQB):
                    et = e_pool.tile([P, KC, QB], F32R, tag="et", name="et")
                    cps = c_psum.tile([dh + 1, QB], F32, tag="cp", name="cp")
                    for cp in range(KC // 2):
                        sps = s_psum.tile([P, 2 * QB], F32, tag="sp", name="sp")
                        for sub in range(2):
                            c = 2 * cp + sub
                            nc.tensor.matmul(
                                sps[:, sub * QB:(sub + 1) * QB],
                                ktile[prow:prow + dh, c * P:(c + 1) * P],
                                qtile[prow:prow + dh, j * QB:(j + 1) * QB],
                                start=True, stop=True)
                        nc.scalar.activation(
                            et[:, 2 * cp:2 * cp + 2, :].rearrange("p a b -> p (a b)"),
                            sps[:],
                            mybir.ActivationFunctionType.Exp,
                            scale=1.0 / SCALE)
                        for sub in range(2):
                            c = 2 * cp + sub
                            nc.tensor.matmul(
                                cps[:],
                                v_aug[:, c, h, :],
                                et[:, c, :],
                                start=(c == 0), stop=(c == KC - 1))
                    nc.vector.tensor_copy(
                        ctxt[h][:, j * QB:(j + 1) * QB], cps[:])
                    nc.sync.dma_start(
                        eT_ap[h].rearrange("(c p) q -> p c q", p=P)
                        [:, :, j * QB:(j + 1) * QB],
                        et[:])

        # ---- phase 3: normalize context, output projection ----
        with ExitStack() as fctx:
            rb_pool = fctx.enter_context(tc.tile_pool(name="rb", bufs=2))
            o_pool = fctx.enter_context(tc.tile_pool(name="osb", bufs=3))
            o_psum = fctx.enter_context(
                tc.tile_pool(name="opsum", bufs=2, space="PSUM"))

            for h in range(heads):
                nc.sync.dma_start(r_ap[h:h + 1, :], ctxt[h][dh:dh + 1, :])
                # partition_broadcast ucode reads the tile's partition 0
                # regardless of the AP's base partition (HW-verified), so
                # DMA the sum row down to partition 0 of a temp tile first.
                rtmp = rb_pool.tile([1, S], F32, tag="rtmp", name="rtmp")
                nc.sync.dma_start(rtmp[:], ctxt[h][dh:dh + 1, :].bitcast(F32))
                rb = rb_pool.tile([dh, S], F32, tag="rb", name="rb")
                nc.gpsimd.partition_broadcast(rb[:], rtmp[:])
                rinv = rb_pool.tile([dh, S], F32, tag="rinv", name="rinv")
                nc.vector.reciprocal(rinv[:], rb[:])
                nc.vector.tensor_mul(
                    ctxt[h][0:dh, :], ctxt[h][0:dh, :].bitcast(F32), rinv[:])

            OB = min(QB, D)
            for st in range(S // P):
                ot = o_pool.tile([P, D], F32, tag="ot", name="ot")
                for nb in range(D // OB):
                    ops = o_psum.tile([P, OB], F32, tag="op", name="op")
                    for h in range(heads):
                        nc.tensor.matmul(
                            ops[:],
                            ctxt[h][0:dh, st * P:(st + 1) * P],
                            wo[h][:, nb * OB:(nb + 1) * OB],
                            start=(h == 0), stop=(h == heads - 1))
                    nc.vector.tensor_copy(ot[:, nb * OB:(nb + 1) * OB], ops[:])
                nc.sync.dma_start(outp_ap[st * P:(st + 1) * P, :], ot[:])

    return nc


_COMPILED = None


def _get_compiled():
    global _COMPILED
    if _COMPILED is None:
        nc = bacc.Bacc("TRN2", target_bir_lowering=False, debug=False,
                       num_devices=N_CORES)
        build_mha_kernel(nc, S_FULL, D_MODEL, HG, HEAD_DIM)
        nc.compile()
        _COMPILED = nc
    return _COMPILED


def kernel(query, key, value, w_q, w_k, w_v, w_o, b_o):
    global LAST_RESULTS
    query = np.ascontiguousarray(np.asarray(query, dtype=np.float32))
    key = np.ascontiguousarray(np.asarray(key, dtype=np.float32))
    value = np.ascontiguousarray(np.asarray(value, dtype=np.float32))
    w_q = np.asarray(w_q, dtype=np.float32)
    w_k = np.asarray(w_k, dtype=np.float32)
    w_v = np.asarray(w_v, dtype=np.float32)
    w_o = np.asarray(w_o, dtype=np.float32)
    b_o = np.asarray(b_o, dtype=np.float32)

    nc = _get_compiled()

    G = HG * HEAD_DIM
    xT = {}
    for b in range(B_FULL):
        xT[b] = (np.ascontiguousarray(query[b].T),
                 np.ascontiguousarray(key[b].T),
                 np.ascontiguousarray(value[b].T))
    wT = {}
    for g in range(N_CORES // B_FULL):
        sl = slice(g * G, (g + 1) * G)
        wT[g] = (np.ascontiguousarray(w_q[sl, :].T),
                 np.ascontiguousarray(w_k[sl, :].T),
                 np.ascontiguousarray(w_v[sl, :].T),
                 np.ascontiguousarray(w_o[:, sl].T))

    in_maps = []
    for c in range(N_CORES):
        b, g = c // 4, c % 4
        xq, xk, xv = xT[b]
        wq, wk, wv, wo = wT[g]
        in_maps.append({"xqT": xq, "xkT": xk, "xvT": xv,
                        "wqT": wq, "wkT": wk, "wvT": wv, "woT": wo})

    res = run_bass_kernel_spmd(nc, in_maps, core_ids=list(range(N_CORES)))
    LAST_RESULTS = res

    out = np.empty((B_FULL, S_FULL, D_MODEL), dtype=np.float32)
    attn = np.empty((B_FULL, NUM_HEADS, S_FULL, S_FULL), dtype=np.float32)
    acc = [None] * B_FULL
    for c in range(N_CORES):
        b, g = c // 4, c % 4
        rc = res.results[c]
        acc[b] = rc["outp"] if acc[b] is None else acc[b] + rc["outp"]
        rinv = 1.0 / rc["r"]                     # [HG, S]
        eT = rc["eT"]                            # [HG, S(k), S(q)]
        for h in range(HG):
            # attn[b, 4g+h, q, k] = eT[h, k, q] * rinv[h, q]
            np.multiply(eT[h].T, rinv[h][:, None], out=attn[b, 4 * g + h])
    for b in range(B_FULL):
        out[b] = acc[b] + b_o
    return out, attn


# revision 6
# speedup vs baseline: 1.0697x; 1.0697x over previous
"""Multi-head attention (B=2, S=2048, d_model=1024, 16 heads) on 8 TRN2 cores.

Sharding: core c = (batch b, head-group g) with b = c // 4, g = c % 4; each core
owns 4 heads of one batch element (Megatron-style column split of w_q/w_k/w_v,
row split of w_o, data parallel over batch).

Device computes, per core:
  - eT[h, k, q]  = exp(scores[q, k] / 8)   (TRANSPOSED, unnormalized)
  - r[h, q]      = sum_k exp(...)          (softmax denominators, via a
                                            ones-column folded into the
                                            context matmul)
  - outp[q, :]   = sum_h (exp/r) @ V_h @ W_o_h.T   (normalized, no bias)

The transposed score layout lets the context matmul consume exp tiles straight
from SBUF with k on partitions (no on-chip transpose, single exp pass).  The
host applies 1/r and transposes eT into the required attn_weights layout while
assembling the full outputs (that pass is needed anyway to gather shards).

All matmuls run as float32r (full-rate); everything else is fp32.
"""

import numpy as np
from contextlib import ExitStack

import concourse.bass as bass
import concourse.bacc as bacc
import concourse.tile as tile
from concourse import mybir
from concourse.bass_utils import run_bass_kernel_spmd

F32 = mybir.dt.float32
F32R = mybir.dt.float32r

D_MODEL = 1024
NUM_HEADS = 16
HEAD_DIM = D_MODEL // NUM_HEADS
B_FULL = 2
S_FULL = 2048
N_CORES = 8
HG = 4                      # heads per core
SCALE = float(np.sqrt(HEAD_DIM))

# stash of the most recent BassKernelResults (test.py reads exec_time_ns)
LAST_RESULTS = None


def _r(ap):
    return ap.bitcast(F32R)


def build_mha_kernel(nc, S, D, heads, dh):
    """Emit the per-core MHA program. Shapes:
      xqT/xkT/xvT: [D, S]   (input activations, pre-transposed on host)
      wqT/wkT/wvT: [D, G]   (G = heads*dh head-group slice, pre-transposed)
      woT:         [G, D]
      eT:  [heads, S, S]  out   (exp scores, transposed per head: [k, q])
      r:   [heads, S]     out   (softmax denominators)
      outp:[S, D]         out   (output-projection partial, normalized)
    """
    P = 128
    QB = 512                 # q-block width (psum free dim)
    G = heads * dh
    KC = S // P              # k chunks of 128
    DC = D // P              # d chunks of 128
    NQB = S // QB            # q blocks
    NSB = S // QB            # s blocks for q/k projections
    NT = G // P              # number of 128-row tiles for qT/kT
    assert G % P == 0 and S % QB == 0 and D % P == 0

    ap = {}
    for name, shape in [("xqT", [D, S]), ("xkT", [D, S]), ("xvT", [D, S]),
                        ("wqT", [D, G]), ("wkT", [D, G]), ("wvT", [D, G]),
                        ("woT", [G, D])]:
        ap[name] = nc.dram_tensor(name, shape, F32R, kind="ExternalInput").ap()
    eT_ap = nc.dram_tensor("eT", [heads, S, S], F32R, kind="ExternalOutput").ap()
    r_ap = nc.dram_tensor("r", [heads, S], F32R, kind="ExternalOutput").ap()
    outp_ap = nc.dram_tensor("outp", [S, D], F32, kind="ExternalOutput").ap()

    with tile.TileContext(nc) as tc, ExitStack() as ctx:
        # ---- persistent pools (live across phases) ----
        qk_pool = ctx.enter_context(tc.tile_pool(name="qk", bufs=1))
        v_pool = ctx.enter_context(tc.tile_pool(name="vaug", bufs=1))
        wo_pool = ctx.enter_context(tc.tile_pool(name="wo", bufs=1))
        ctx_pool = ctx.enter_context(tc.tile_pool(name="ctx", bufs=1))

        qt = [qk_pool.tile([P, S], F32R, tag=f"qt{m}", name=f"qt{m}") for m in range(NT)]
        kt = [qk_pool.tile([P, S], F32R, tag=f"kt{m}", name=f"kt{m}") for m in range(NT)]
        v_aug = v_pool.tile([P, KC, heads, dh + 1], F32R)
        wo = [wo_pool.tile([dh, D], F32R, tag=f"wo{h}", name=f"wo{h}") for h in range(heads)]
        ctxt = [ctx_pool.tile([dh + 1, S], F32R, tag=f"ctx{h}", name=f"ctx{h}") for h in range(heads)]

        for h in range(heads):
            nc.sync.dma_start(wo[h][:], ap["woT"][h * dh:(h + 1) * dh, :])
        # ones column for the context-matmul row-sum trick (memset cannot
        # write fp32r, so memset an fp32 tile and let a DVE copy round it)
        ones = v_pool.tile([P, KC * heads], F32)
        nc.gpsimd.memset(ones[:], 1.0)
        nc.vector.tensor_copy(
            v_aug[:, :, :, dh:dh + 1],
            ones[:].rearrange("p (c h o) -> p c h o", c=KC, h=heads, o=1))

        # ---- phase 1: projections ----
        with ExitStack() as pctx:
            w_pool = pctx.enter_context(tc.tile_pool(name="wqkv", bufs=1))
            x_pool = pctx.enter_context(tc.tile_pool(name="xt", bufs=10))
            pj_psum = pctx.enter_context(
                tc.tile_pool(name="pjpsum", bufs=4, space="PSUM"))

            wts = {}
            for wname in ("wqT", "wkT", "wvT"):
                wt = w_pool.tile([P, DC, G], F32R, tag=wname, name=wname)
                nc.sync.dma_start(
                    wt[:], ap[wname].rearrange("(c p) n -> p c n", p=P))
                wts[wname] = wt

            def load_x(xname):
                tiles = []
                for c in range(DC):
                    t = x_pool.tile([P, S], F32R, tag="xc", name="xc")
                    nc.sync.dma_start(t[:], ap[xname][c * P:(c + 1) * P, :])
                    tiles.append(t)
                return tiles

            # q/k projections -> qt/kt in [dims, s] layout
            for wname, dst in (("wqT", qt), ("wkT", kt)):
                xts = load_x("xqT" if wname == "wqT" else "xkT")
                for m in range(NT):
                    for jb in range(NSB):
                        ps = pj_psum.tile([P, QB], F32, tag="pj", name="pj")
                        for c in range(DC):
                            nc.tensor.matmul(
                                ps[:],
                                wts[wname][:, c, m * P:(m + 1) * P],
                                xts[c][:, jb * QB:(jb + 1) * QB],
                                start=(c == 0), stop=(c == DC - 1))
                        nc.vector.tensor_copy(
                            dst[m][:, jb * QB:(jb + 1) * QB], ps[:])

            # v projection -> v_aug in [s, dims] layout (natural)
            xts = load_x("xvT")
            for i in range(KC):
                ps = pj_psum.tile([P, G], F32, tag="pv", name="pv")
                for c in range(DC):
                    nc.tensor.matmul(
                        ps[:],
                        xts[c][:, i * P:(i + 1) * P],
                        wts["wvT"][:, c, :],
                        start=(c == 0), stop=(c == DC - 1))
                for h in range(heads):
                    nc.vector.tensor_copy(
                        v_aug[:, i, h, 0:dh], ps[:, h * dh:(h + 1) * dh])

        # ---- phase 2+3: attention, q-block-major, finale fused per block ----
        # For each q block: all heads' scores/exp/eT-store/context, then the
        # per-block softmax normalization and output projection for those q
        # rows.  The DMA stream (the bottleneck) stays dense and the finale
        # overlaps the next block's attention instead of serializing at the
        # end.
        HB = KC // 2             # k chunks per half eT tile
        OB = min(QB, D)
        SPB = QB // P            # s tiles per q block (out-proj rows)
        with ExitStack() as actx:
            e_pool = actx.enter_context(tc.tile_pool(name="et", bufs=4))
            rb_pool = actx.enter_context(tc.tile_pool(name="rb", bufs=2))
            o_pool = actx.enter_context(tc.tile_pool(name="osb", bufs=3))
            s_psum = actx.enter_context(
                tc.tile_pool(name="spsum", bufs=2, space="PSUM"))
            c_psum = actx.enter_context(
                tc.tile_pool(name="cpsum", bufs=2, space="PSUM"))
            o_psum = actx.enter_context(
                tc.tile_pool(name="opsum", bufs=2, space="PSUM"))

            for j in range(NQB):
                for h in range(heads):
                    m, prow = h // 2, (h % 2) * dh if dh < P else 0
                    ktile, qtile = kt[m], qt[m]
                    cps = c_psum.tile([dh + 1, QB], F32, tag="cp", name="cp")
                    for half in range(2):
                        eth = e_pool.tile([P, HB, QB], F32R, tag="et", name="et")
                        for cp in range(HB // 2):
                            sps = s_psum.tile([P, 2 * QB], F32, tag="sp",
                                              name="sp")
                            for sub in range(2):
                                c = half * HB + 2 * cp + sub
                                nc.tensor.matmul(
                                    sps[:, sub * QB:(sub + 1) * QB],
                                    ktile[prow:prow + dh, c * P:(c + 1) * P],
                                    qtile[prow:prow + dh,
                                          j * QB:(j + 1) * QB],
                                    start=True, stop=True)
                            nc.scalar.activation(
                                eth[:, 2 * cp:2 * cp + 2, :]
                                .rearrange("p a b -> p (a b)"),
                                sps[:],
                                mybir.ActivationFunctionType.Exp,
                                scale=1.0 / SCALE)
                            for sub in range(2):
                                c = half * HB + 2 * cp + sub
                                nc.tensor.matmul(
                                    cps[:],
                                    v_aug[:, c, h, :],
                                    eth[:, 2 * cp + sub, :],
                                    start=(c == 0), stop=(c == KC - 1))
                        nc.sync.dma_start(
                            eT_ap[h].rearrange("(c p) q -> p c q", p=P)
                            [:, half * HB:(half + 1) * HB,
                             j * QB:(j + 1) * QB],
                            eth[:])
                    nc.vector.tensor_copy(
                        ctxt[h][:, j * QB:(j + 1) * QB], cps[:])

                # per-block finale: softmax denominators + normalization
                jsl = slice(j * QB, (j + 1) * QB)
                for h in range(heads):
                    nc.sync.dma_start(r_ap[h:h + 1, jsl],
                                      ctxt[h][dh:dh + 1, jsl])
                    # partition_broadcast ucode reads the tile's partition 0
                    # regardless of the AP's base partition (HW-verified), so
                    # DMA the sum row down to partition 0 of a temp tile.
                    rtmp = rb_pool.tile([1, QB], F32, tag="rtmp", name="rtmp")
                    nc.sync.dma_start(rtmp[:],
                                      ctxt[h][dh:dh + 1, jsl].bitcast(F32))
                    rinv = rb_pool.tile([1, QB], F32, tag="rinv", name="rinv")
                    nc.vector.reciprocal(rinv[:], rtmp[:])
                    rb = rb_pool.tile([dh, QB], F32, tag="rb", name="rb")
                    nc.gpsimd.partition_broadcast(rb[:], rinv[:])
                    nc.vector.tensor_mul(
                        ctxt[h][0:dh, jsl],
                        ctxt[h][0:dh, jsl].bitcast(F32), rb[:])

                # per-block output projection (s rows of this q block)
                for sti in range(SPB):
                    st = j * SPB + sti
                    ot = o_pool.tile([P, D], F32, tag="ot", name="ot")
                    for nb in range(D // OB):
                        ops = o_psum.tile([P, OB], F32, tag="op", name="op")
                        for h in range(heads):
                            nc.tensor.matmul(
                                ops[:],
                                ctxt[h][0:dh, st * P:(st + 1) * P],
                                wo[h][:, nb * OB:(nb + 1) * OB],
                                start=(h == 0), stop=(h == heads - 1))
                        nc.vector.tensor_copy(
                            ot[:, nb * OB:(nb + 1) * OB], ops[:])
                    nc.sync.dma_start(outp_ap[st * P:(st + 1) * P, :], ot[:])

    return nc


_COMPILED = None


def _get_compiled():
    global _COMPILED
    if _COMPILED is None:
        nc = bacc.Bacc("TRN2", target_bir_lowering=False, debug=False,
                       num_devices=N_CORES)
        build_mha_kernel(nc, S_FULL, D_MODEL, HG, HEAD_DIM)
        nc.compile()
        _COMPILED = nc
    return _COMPILED


def kernel(query, key, value, w_q, w_k, w_v, w_o, b_o):
    global LAST_RESULTS
    query = np.ascontiguousarray(np.asarray(query, dtype=np.float32))
    key = np.ascontiguousarray(np.asarray(key, dtype=np.float32))
    value = np.ascontiguousarray(np.asarray(value, dtype=np.float32))
    w_q = np.asarray(w_q, dtype=np.float32)
    w_k = np.asarray(w_k, dtype=np.float32)
    w_v = np.asarray(w_v, dtype=np.float32)
    w_o = np.asarray(w_o, dtype=np.float32)
    b_o = np.asarray(b_o, dtype=np.float32)

    nc = _get_compiled()

    G = HG * HEAD_DIM
    xT = {}
    for b in range(B_FULL):
        xT[b] = (np.ascontiguousarray(query[b].T),
                 np.ascontiguousarray(key[b].T),
                 np.ascontiguousarray(value[b].T))
    wT = {}
    for g in range(N_CORES // B_FULL):
        sl = slice(g * G, (g + 1) * G)
        wT[g] = (np.ascontiguousarray(w_q[sl, :].T),
                 np.ascontiguousarray(w_k[sl, :].T),
                 np.ascontiguousarray(w_v[sl, :].T),
                 np.ascontiguousarray(w_o[:, sl].T))

    in_maps = []
    for c in range(N_CORES):
        b, g = c // 4, c % 4
        xq, xk, xv = xT[b]
        wq, wk, wv, wo = wT[g]
        in_maps.append({"xqT": xq, "xkT": xk, "xvT": xv,
                        "wqT": wq, "wkT": wk, "wvT": wv, "woT": wo})

    res = run_bass_kernel_spmd(nc, in_maps, core_ids=list(range(N_CORES)))
    LAST_RESULTS = res

    out = np.empty((B_FULL, S_FULL, D_MODEL), dtype=np.float32)
    attn = np.empty((B_FULL, NUM_HEADS, S_FULL, S_FULL), dtype=np.float32)
    acc = [None] * B_FULL
    for c in range(N_CORES):
        b, g = c // 4, c % 4
        rc = res.results[c]
        acc[b] = rc["outp"] if acc[b] is None else acc[b] + rc["outp"]
        rinv = 1.0 / rc["r"]                     # [HG, S]
        eT = rc["eT"]                            # [HG, S(k), S(q)]
        for h in range(HG):
            # attn[b, 4g+h, q, k] = eT[h, k, q] * rinv[h, q]
            np.multiply(eT[h].T, rinv[h][:, None], out=attn[b, 4 * g + h])
    for b in range(B_FULL):
        out[b] = acc[b] + b_o
    return out, attn
